# revision 1
# baseline (speedup 1.0000x reference)
"""Mutual channel attention (sparse_attention) TRN2 Bass kernel.

Problem: x1, x2 of shape (16, 512, 64, 64) fp32.
  q = x1.reshape(B, C, D), k = x2.reshape(B, C, D), D = 4096, scale = 1/64
  S   = q @ k^T * scale                       [B, 512, 512]
  outA = softmax_rows(S) @ k                  -> (16, 512, 64, 64)
  outB = softmax_rows(S^T) @ q                -> (16, 512, 64, 64)

Key algebra: without max-subtraction (scores ~ N(0,1), safe in fp32),
P = exp(S*scale) serves BOTH directions; only the normalization sums
differ (row sums of P for A, column sums of P for B).
  outA[c,:] = (P @ k)[c,:]   / rowsum_P[c]
  outB[e,:] = (P^T @ q)[e,:] / colsum_P[e]

Sharding: pure data parallel, 2 batches per core across 8 cores.

All matmuls run in float32r (single-pass fp32, 1 cycle/row at N=512,
~2e-4 rel err vs the fp32 reference on HW). q/k live in SBUF as 4x4
quarter tiles [128, 1024] so slots free progressively during the
d-outer out phase and the next batch's loads overlap compute.

Per-core per-batch schedule:
  1. Load q,k quarter tiles (quarter-major: the scores phase can start
     after the first 4.2MB lands).
  2. Scores: per 128-wide d-chunk, PE-transpose 4 q-blocks + 4
     k-blocks into [128,512] psum staging, copy to SBUF (q-half on
     DVE, k-half on ACT), 4 accumulating matmuls into resident S banks.
  3. exp via ScalarE with fused *1/64 scale and fused row-sum.
  4. PE-transpose P -> P_ec with fused column-sum on the copy-out.
  5. out_a = P_ec.T @ k (d-outer, frees k quarters early for the next
     batch's k loads), then out_b = P_ce.T @ q (same for q);
     normalization folded into the PSUM->SBUF copy as a per-partition
     scale; copies alternate DVE/ACT.
"""

import numpy as np

B, C, D = 16, 512, 4096
N_CORES = 8
B_PER_CORE = B // N_CORES  # 2
CC = C // 128  # 4 c-chunks
DC = D // 128  # 32 d-chunks
NQ = 8  # d-slices per row-chunk tile ([128,512] eighths: halves first-load wait, finer frees)
QW = D // NQ  # 1024 quarter width
NG = D // 512  # 8 d-groups of 512 in the out phase

_COMPILED = {}


def _build():
    import concourse.mybir as mybir
    from concourse import bacc, tile

    f32 = mybir.dt.float32
    f32r = mybir.dt.float32r
    bf16 = mybir.dt.bfloat16
    AF = mybir.ActivationFunctionType
    ROWS = B_PER_CORE * C  # 1024

    nc = bacc.Bacc(None, target_bir_lowering=False)
    x1 = nc.declare_dram_parameter("x1", [ROWS, D], f32r, isOutput=False)
    x2 = nc.declare_dram_parameter("x2", [ROWS, D], f32r, isOutput=False)
    ident = nc.declare_dram_parameter("ident", [128, 128], f32r, isOutput=False)
    outA = nc.declare_dram_parameter("outA", [ROWS, D], f32, isOutput=True)
    outB = nc.declare_dram_parameter("outB", [ROWS, D], f32, isOutput=True)

    with tile.TileContext(nc) as tc:
        with (
            tc.tile_pool(name="const", bufs=1) as constp,
            tc.tile_pool(name="qk", bufs=1) as qk,
            tc.tile_pool(name="stg_sb", bufs=3) as stgsb,
            tc.tile_pool(name="pp", bufs=2) as pp,
            tc.tile_pool(name="rp", bufs=2) as rp,
            tc.tile_pool(name="osb", bufs=6) as osb,
            tc.tile_pool(name="sps", bufs=1, space="PSUM") as sps,
            tc.tile_pool(name="stgps", bufs=4, space="PSUM") as stgps,
        ):
            idt = constp.tile([128, 128], f32r)
            nc.sync.dma_start(idt[:], ident[:])

            for b in range(B_PER_CORE):
                r0 = b * C
                # ---- load q, k as quarter tiles, quarter-major ----
                q = [[None] * NQ for _ in range(CC)]
                k = [[None] * NQ for _ in range(CC)]
                for h in range(NQ):
                    for cc in range(CC):
                        rows = slice(r0 + cc * 128, r0 + (cc + 1) * 128)
                        cols = slice(h * QW, (h + 1) * QW)
                        qt = qk.tile(
                            [128, QW], f32r, tag=f"q{cc}_{h}", name=f"q{cc}_{h}"
                        )
                        kt = qk.tile(
                            [128, QW], f32r, tag=f"k{cc}_{h}", name=f"k{cc}_{h}"
                        )
                        nc.sync.dma_start(qt[:], x1[rows, cols])
                        nc.sync.dma_start(kt[:], x2[rows, cols])
                        q[cc][h] = qt
                        k[cc][h] = kt

                # ---- scores: S_ce[cc] accumulates over 32 d-chunks ----
                s_ps = [
                    sps.tile([128, C], f32, tag=f"s{cc}", name=f"s{cc}")
                    for cc in range(CC)
                ]
                for dc in range(DC):
                    h, off = divmod(dc * 128, QW)
                    dsl = slice(off, off + 128)
                    qt_ps = stgps.tile([128, 512], f32r, tag="st", name="qt_ps")
                    kt_ps = stgps.tile([128, 512], f32r, tag="st", name="kt_ps")
                    for cc in range(CC):
                        csl = slice(cc * 128, (cc + 1) * 128)
                        nc.tensor.transpose(qt_ps[:, csl], q[cc][h][:, dsl], idt[:])
                        nc.tensor.transpose(kt_ps[:, csl], k[cc][h][:, dsl], idt[:])
                    qt_sb = stgsb.tile([128, 512], f32r, tag="qt_sb", name="qt_sb")
                    kt_sb = stgsb.tile([128, 512], f32r, tag="kt_sb", name="kt_sb")
                    nc.vector.tensor_copy(qt_sb[:], qt_ps[:])
                    nc.scalar.activation(kt_sb[:], kt_ps[:], AF.Copy)
                    for cc in range(CC):
                        nc.tensor.matmul(
                            s_ps[cc][:],
                            qt_sb[:, cc * 128 : (cc + 1) * 128],
                            kt_sb[:],
                            start=(dc == 0),
                            stop=(dc == DC - 1),
                        )

                # ---- exp + row sums (direction A) ----
                p_ce = []
                rinv_a = []
                for cc in range(CC):
                    p = pp.tile([128, C], f32r, tag=f"pce{cc}", name=f"pce{cc}")
                    rs = rp.tile([128, 1], f32, tag=f"rsa{cc}", name=f"rsa{cc}")
                    nc.scalar.activation(
                        p[:], s_ps[cc][:], AF.Exp, scale=1.0 / 64.0, accum_out=rs[:]
                    )
                    ri = rp.tile([128, 1], f32, tag=f"ria{cc}", name=f"ria{cc}")
                    nc.vector.reciprocal(ri[:], rs[:])
                    p_ce.append(p)
                    rinv_a.append(ri)

                # ---- transpose P -> P_ec + column sums (direction B) ----
                p_ec = []
                rinv_b = []
                for ec in range(CC):
                    esl = slice(ec * 128, (ec + 1) * 128)
                    t_ps = stgps.tile([128, 512], f32r, tag="st", name="pt_ps")
                    for cc in range(CC):
                        nc.tensor.transpose(
                            t_ps[:, cc * 128 : (cc + 1) * 128], p_ce[cc][:, esl], idt[:]
                        )
                    p = pp.tile([128, C], f32r, tag=f"pec{ec}", name=f"pec{ec}")
                    rs = rp.tile([128, 1], f32, tag=f"rsb{ec}", name=f"rsb{ec}")
                    nc.scalar.activation(p[:], t_ps[:], AF.Copy, accum_out=rs[:])
                    ri = rp.tile([128, 1], f32, tag=f"rib{ec}", name=f"rib{ec}")
                    nc.vector.reciprocal(ri[:], rs[:])
                    p_ec.append(p)
                    rinv_b.append(ri)

                # ---- out_a = (P_ec.T @ k) * rinv_a, d-outer frees k early ----
                for g in range(NG):
                    h, off = divmod(g * 512, QW)
                    dsl = slice(off, off + 512)
                    for cc in range(CC):
                        csl = slice(cc * 128, (cc + 1) * 128)
                        o_ps = stgps.tile([128, 512], f32, tag="st", name="oa_ps")
                        for ec in range(CC):
                            nc.tensor.matmul(
                                o_ps[:],
                                p_ec[ec][:, csl],
                                k[ec][h][:, dsl],
                                start=(ec == 0),
                                stop=(ec == CC - 1),
                            )
                        o_sb = osb.tile([128, 512], f32, tag="osb", name="oa_sb")
                        if cc % 2 == 0:
                            nc.vector.tensor_scalar_mul(o_sb[:], o_ps[:], rinv_a[cc][:])
                        else:
                            nc.scalar.activation(
                                o_sb[:], o_ps[:], AF.Copy, scale=rinv_a[cc][:]
                            )
                        nc.sync.dma_start(
                            outA[
                                r0 + cc * 128 : r0 + (cc + 1) * 128,
                                g * 512 : (g + 1) * 512,
                            ],
                            o_sb[:],
                        )

                # ---- out_b = (P_ce.T @ q) * rinv_b, d-outer frees q early ----
                for g in range(NG):
                    h, off = divmod(g * 512, QW)
                    dsl = slice(off, off + 512)
                    for ec in range(CC):
                        esl = slice(ec * 128, (ec + 1) * 128)
                        o_ps = stgps.tile([128, 512], f32, tag="st", name="ob_ps")
                        for cc in range(CC):
                            nc.tensor.matmul(
                                o_ps[:],
                                p_ce[cc][:, esl],
                                q[cc][h][:, dsl],
                                start=(cc == 0),
                                stop=(cc == CC - 1),
                            )
                        o_sb = osb.tile([128, 512], f32, tag="osb", name="ob_sb")
                        if ec % 2 == 0:
                            nc.vector.tensor_scalar_mul(o_sb[:], o_ps[:], rinv_b[ec][:])
                        else:
                            nc.scalar.activation(
                                o_sb[:], o_ps[:], AF.Copy, scale=rinv_b[ec][:]
                            )
                        nc.sync.dma_start(
                            outB[
                                r0 + ec * 128 : r0 + (ec + 1) * 128,
                                g * 512 : (g + 1) * 512,
                            ],
                            o_sb[:],
                        )

    nc.finalize()
    return nc


def _get_nc():
    if "nc" not in _COMPILED:
        _COMPILED["nc"] = _build()
    return _COMPILED["nc"]


def kernel(x1: np.ndarray, x2: np.ndarray):
    from concourse.bass_utils import run_bass_kernel_spmd

    nc = _get_nc()
    x1 = np.ascontiguousarray(x1, dtype=np.float32)
    x2 = np.ascontiguousarray(x2, dtype=np.float32)
    ident = np.eye(128, dtype=np.float32)

    in_maps = []
    for i in range(N_CORES):
        sl = slice(i * B_PER_CORE, (i + 1) * B_PER_CORE)
        in_maps.append(
            {
                "x1": x1[sl].reshape(B_PER_CORE * C, D),
                "x2": x2[sl].reshape(B_PER_CORE * C, D),
                "ident": ident,
            }
        )

    res = None
    for attempt in range(3):
        try:
            res = run_bass_kernel_spmd(nc, in_maps, list(range(N_CORES))).results
            break
        except Exception:
            if attempt == 2:
                raise
    assert res is not None

    outA = np.empty((B, C, 64, 64), dtype=np.float32)
    outB = np.empty((B, C, 64, 64), dtype=np.float32)
    for i in range(N_CORES):
        sl = slice(i * B_PER_CORE, (i + 1) * B_PER_CORE)
        outA[sl] = res[i]["outA"].reshape(B_PER_CORE, C, 64, 64)
        outB[sl] = res[i]["outB"].reshape(B_PER_CORE, C, 64, 64)
    return outA, outB



# revision 2
# speedup vs baseline: 1.0178x; 1.0178x over previous
"""Mutual channel attention (sparse_attention) TRN2 Bass kernel,.

See kernel_v2 docstring for the core design (dual-layout bf16 inputs,
transpose-free PE, bf16 stores upcast on host). v3 reduces the
batch-0 DMA-bound fill and the tail overheads:
  - d-major loads merged: qt+kt group g in ONE 2MB DMA (host-packed).
  - channel-major q, k each ONE 4MB DMA (host-packed p-major).
  - load order per batch: qkt groups, then qc (out_b needs it first),
    then kc.
  - row/col-sum + reciprocal tiles consolidated to [128,4] (fewer
    semaphores -> shorter BSP epilogue; 2 reciprocals instead of 8).
  - the final store of the last batch is split in half so the end
    drain is ~1.5us instead of ~3us.
"""

import numpy as np

B, C, D = 16, 512, 4096
N_CORES = 8
B_PER_CORE = B // N_CORES  # 2
CC = C // 128  # 4 c-chunks
DC = D // 128  # 32 d-chunks
NG = D // 512  # 8 d-groups of 512 in the out phase
GQ = 8  # d-chunks per d-major load group
NGT = DC // GQ  # 4 d-major groups
GW = GQ * C  # 4096: per-tensor width of one d-major group
ROWS = B_PER_CORE * C  # 1024

_COMPILED = {}


def _build():
    import concourse.mybir as mybir
    from concourse import bacc, tile

    f32 = mybir.dt.float32
    bf16 = mybir.dt.bfloat16
    AF = mybir.ActivationFunctionType

    nc = bacc.Bacc(None, target_bir_lowering=False)
    # qkt: d-major q and k interleaved per group g:
    #   row b*128+p, col g*(2*GW) + half*GW + j*C + c  (half 0 = q, 1 = k)
    #   holds element (d = 128*(g*GQ+j)+p, c) of q/k for batch b.
    # qc/kc: channel-major, p-major packed:
    #   row b*128+p, col cc*D + d  ->  q[cc*128+p, d] of batch b.
    qkt = nc.declare_dram_parameter(
        "qkt", [B_PER_CORE * 128, NGT * 2 * GW], bf16, isOutput=False
    )
    qc = nc.declare_dram_parameter("qc", [B_PER_CORE * 128, CC * D], bf16, isOutput=False)
    kc = nc.declare_dram_parameter("kc", [B_PER_CORE * 128, CC * D], bf16, isOutput=False)
    ident = nc.declare_dram_parameter("ident", [128, 128], bf16, isOutput=False)
    oa = nc.declare_dram_parameter("oa", [ROWS, D], bf16, isOutput=True)
    ob = nc.declare_dram_parameter("ob", [ROWS, D], bf16, isOutput=True)

    with tile.TileContext(nc) as tc:
        with (
            tc.tile_pool(name="const", bufs=1) as constp,
            tc.tile_pool(name="qk", bufs=1) as qk,
            tc.tile_pool(name="pp", bufs=1) as pp,
            tc.tile_pool(name="rp", bufs=1) as rp,
            tc.tile_pool(name="osb", bufs=3) as osb,
            tc.tile_pool(name="sps", bufs=1, space="PSUM") as sps,
            tc.tile_pool(name="stgps", bufs=4, space="PSUM") as stgps,
        ):
            idt = constp.tile([128, 128], bf16)
            nc.sync.dma_start(idt[:], ident[:])

            def load_qkt(b):
                """Everything rides the single SP HWDGE ring, issued in the
                order it becomes loadable, so DMA-sem recycle waits land on
                the (idle) Sync queue and never block ACT/DVE compute."""
                p0 = b * 128
                qkt_t = []
                for g in range(NGT):
                    a = qk.tile([128, 2 * GW], bf16, tag=f"qkt{g}", name=f"qkt{g}")
                    nc.sync.dma_start(
                        a[:], qkt[p0 : p0 + 128, g * 2 * GW : (g + 1) * 2 * GW]
                    )
                    qkt_t.append(a)
                return qkt_t

            def load_c(b, dram, pfx):
                p0 = b * 128
                out = []
                for cc in range(CC):
                    a = qk.tile([128, D], bf16, tag=f"{pfx}{cc}", name=f"{pfx}{cc}")
                    nc.sync.dma_start(a[:], dram[p0 : p0 + 128, cc * D : (cc + 1) * D])
                    out.append(a)
                return out

            # HAM warm-up: ~44 junk matmuls on the identity keep the PE
            # busy through the DMA-bound fill so the real score matmuls
            # start at 2.4 GHz instead of 1.2.
            junk = stgps.tile([128, 512], f32, tag="st", name="junk")
            for _ in range(44):
                nc.tensor.matmul(junk[:, 0:128], idt[:], idt[:], start=True, stop=True)

            tiles = (load_qkt(0), load_c(0, qc, "qc"), load_c(0, kc, "kc"))
            for b in range(B_PER_CORE):
                qkt_t, qc_t, kc_t = tiles
                r0 = b * C

                # ---- scores: S_ce[cc] accumulates over 32 d-chunks ----
                s_ps = [
                    sps.tile([128, C], f32, tag=f"s{cc}", name=f"s{cc}")
                    for cc in range(CC)
                ]
                # groups 0..NGT-2: d-outer (stream-friendly while loads
                # land); last group: cc-outer so the four S banks complete
                # STAGGERED and the serial exp+rowsum chain on ACT overlaps
                # the score-matmul tail instead of stalling out_b.
                for dc in range(DC - GQ):
                    g, j = divmod(dc, GQ)
                    mv = qkt_t[g][:, GW + j * C : GW + (j + 1) * C]
                    for cc in range(CC):
                        nc.tensor.matmul(
                            s_ps[cc][:],
                            qkt_t[g][:, j * C + cc * 128 : j * C + (cc + 1) * 128],
                            mv,
                            start=(dc == 0),
                            stop=False,
                        )
                for cc in range(CC):
                    for dc in range(DC - GQ, DC):
                        g, j = divmod(dc, GQ)
                        nc.tensor.matmul(
                            s_ps[cc][:],
                            qkt_t[g][:, j * C + cc * 128 : j * C + (cc + 1) * 128],
                            qkt_t[g][:, GW + j * C : GW + (j + 1) * C],
                            start=False,
                            stop=(dc == DC - 1),
                        )

                # next batch's qkt: issued here (ring position after this
                # batch's loads) so its WAR deps release during our scores.
                if b + 1 < B_PER_CORE:
                    next_qkt = load_qkt(b + 1)

                # ---- exp + row sums (direction A) ----
                rs_a = rp.tile([128, CC], f32, tag="rsa", name="rsa")
                ri_a = rp.tile([128, CC], f32, tag="ria", name="ria")
                p_ce = []
                for cc in range(CC):
                    p = pp.tile([128, C], bf16, tag=f"pce{cc}", name=f"pce{cc}")
                    nc.scalar.activation(
                        p[:],
                        s_ps[cc][:],
                        AF.Exp,
                        scale=1.0 / 64.0,
                        accum_out=rs_a[:, cc : cc + 1],
                    )
                    p_ce.append(p)
                nc.vector.reciprocal(ri_a[:], rs_a[:])

                # ---- transpose P -> P_ec + column sums (direction B) ----
                # t_ps reuses the S banks (tags s0-3): free after the exps.
                rs_b = rp.tile([128, CC], f32, tag="rsb", name="rsb")
                ri_b = rp.tile([128, CC], f32, tag="rib", name="rib")
                p_ec = []
                for ec in range(CC):
                    esl = slice(ec * 128, (ec + 1) * 128)
                    t_ps = sps.tile([128, C], bf16, tag=f"s{ec}", name=f"pt{ec}")
                    for cc in range(CC):
                        nc.tensor.transpose(
                            t_ps[:, cc * 128 : (cc + 1) * 128], p_ce[cc][:, esl], idt[:]
                        )
                    p = pp.tile([128, C], bf16, tag=f"pec{ec}", name=f"pec{ec}")
                    nc.scalar.activation(
                        p[:], t_ps[:], AF.Copy, accum_out=rs_b[:, ec : ec + 1]
                    )
                    p_ec.append(p)
                nc.vector.reciprocal(ri_b[:], rs_b[:])

                # ---- out_b = (P_ce.T @ q) * rinv_b ----
                # (before out_a: only needs P_ce, so it hides the P_ec
                # transpose/copy tail.)
                for ec in range(CC):
                    esl = slice(ec * 128, (ec + 1) * 128)
                    o_sb = osb.tile([128, D], bf16, tag="osb", name=f"ob{ec}")
                    for g in range(NG):
                        gsl = slice(g * 512, (g + 1) * 512)
                        o_ps = stgps.tile([128, 512], f32, tag="st", name="ob_ps")
                        for cc in range(CC):
                            nc.tensor.matmul(
                                o_ps[:],
                                p_ce[cc][:, esl],
                                qc_t[cc][:, g * 512 : (g + 1) * 512],
                                start=(cc == 0),
                                stop=(cc == CC - 1),
                            )
                        if g % 2 == 0:
                            nc.vector.tensor_scalar_mul(
                                o_sb[:, gsl], o_ps[:], ri_b[:, ec : ec + 1]
                            )
                        else:
                            nc.scalar.activation(
                                o_sb[:, gsl],
                                o_ps[:],
                                AF.Copy,
                                scale=ri_b[:, ec : ec + 1],
                            )
                    nc.sync.dma_start(
                        ob[r0 + ec * 128 : r0 + (ec + 1) * 128, :], o_sb[:]
                    )

                if b + 1 < B_PER_CORE:
                    next_qc = load_c(b + 1, qc, "qc")

                # ---- out_a = (P_ec.T @ k) * rinv_a ----
                for cc in range(CC):
                    csl = slice(cc * 128, (cc + 1) * 128)
                    o_sb = osb.tile([128, D], bf16, tag="osb", name=f"oa{cc}")
                    for g in range(NG):
                        gsl = slice(g * 512, (g + 1) * 512)
                        o_ps = stgps.tile([128, 512], f32, tag="st", name="oa_ps")
                        for ec in range(CC):
                            nc.tensor.matmul(
                                o_ps[:],
                                p_ec[ec][:, csl],
                                kc_t[ec][:, g * 512 : (g + 1) * 512],
                                start=(ec == 0),
                                stop=(ec == CC - 1),
                            )
                        if g % 2 == 0:
                            nc.vector.tensor_scalar_mul(
                                o_sb[:, gsl], o_ps[:], ri_a[:, cc : cc + 1]
                            )
                        else:
                            nc.scalar.activation(
                                o_sb[:, gsl],
                                o_ps[:],
                                AF.Copy,
                                scale=ri_a[:, cc : cc + 1],
                            )
                        if b == B_PER_CORE - 1 and cc == CC - 1:
                            # stream the final tile out per-group: short drain
                            nc.sync.dma_start(
                                oa[r0 + cc * 128 : r0 + (cc + 1) * 128, gsl],
                                o_sb[:, gsl],
                            )
                    if not (b == B_PER_CORE - 1 and cc == CC - 1):
                        orows = slice(r0 + cc * 128, r0 + (cc + 1) * 128)
                        nc.sync.dma_start(oa[orows, :], o_sb[:])

                if b + 1 < B_PER_CORE:
                    next_kc = load_c(b + 1, kc, "kc")
                    tiles = (next_qkt, next_qc, next_kc)

    nc.finalize()
    return nc


def _get_nc():
    if "nc" not in _COMPILED:
        _COMPILED["nc"] = _build()
    return _COMPILED["nc"]


def _make_in_maps(x1, x2):
    import ml_dtypes

    bf16 = ml_dtypes.bfloat16
    x1 = np.ascontiguousarray(x1, dtype=np.float32).reshape(B, C, D)
    x2 = np.ascontiguousarray(x2, dtype=np.float32).reshape(B, C, D)
    ident = np.eye(128, dtype=bf16)

    in_maps = []
    for i in range(N_CORES):
        sl = slice(i * B_PER_CORE, (i + 1) * B_PER_CORE)
        c1 = x1[sl].astype(bf16)  # [2, C, D]
        c2 = x2[sl].astype(bf16)
        # d-major [b, p, j, c] with d = 128*j + p, then groups of GQ j's
        # with q/k halves interleaved per group.
        t1 = c1.transpose(0, 2, 1).reshape(B_PER_CORE, NGT, GQ, 128, C)
        t2 = c2.transpose(0, 2, 1).reshape(B_PER_CORE, NGT, GQ, 128, C)
        # -> [b, p, g, half, j, c]
        tt = np.stack([t1, t2], axis=2)  # [b, g, half, j, p, c]
        tt = tt.transpose(0, 4, 1, 2, 3, 5)  # [b, p, g, half, j, c]
        qkt_h = np.ascontiguousarray(tt).reshape(B_PER_CORE * 128, NGT * 2 * GQ * C)
        # channel-major p-major: [b, p, cc, d]
        cc1 = c1.reshape(B_PER_CORE, CC, 128, D).transpose(0, 2, 1, 3)
        cc2 = c2.reshape(B_PER_CORE, CC, 128, D).transpose(0, 2, 1, 3)
        in_maps.append(
            {
                "qkt": qkt_h,
                "qc": np.ascontiguousarray(cc1).reshape(B_PER_CORE * 128, CC * D),
                "kc": np.ascontiguousarray(cc2).reshape(B_PER_CORE * 128, CC * D),
                "ident": ident,
            }
        )
    return in_maps


def kernel(x1: np.ndarray, x2: np.ndarray):
    from concourse.bass_utils import run_bass_kernel_spmd

    nc = _get_nc()
    in_maps = _make_in_maps(x1, x2)

    res = None
    for attempt in range(3):
        try:
            res = run_bass_kernel_spmd(nc, in_maps, list(range(N_CORES))).results
            break
        except Exception:
            if attempt == 2:
                raise
    assert res is not None

    outA = np.empty((B, C, 64, 64), dtype=np.float32)
    outB = np.empty((B, C, 64, 64), dtype=np.float32)
    for i in range(N_CORES):
        sl = slice(i * B_PER_CORE, (i + 1) * B_PER_CORE)
        outA[sl] = res[i]["oa"].astype(np.float32).reshape(B_PER_CORE, C, 64, 64)
        outB[sl] = res[i]["ob"].astype(np.float32).reshape(B_PER_CORE, C, 64, 64)
    return outA, outB


# revision 3
# speedup vs baseline: 1.0184x; 1.0006x over previous
"""Mutual channel attention (sparse_attention) TRN2 Bass kernel, v8.

See kernel_v2 docstring for the core design (dual-layout bf16 inputs,
transpose-free PE, bf16 stores upcast on host). v3 reduces the
batch-0 DMA-bound fill and the tail overheads:
  - d-major loads merged: qt+kt group g in ONE 2MB DMA (host-packed).
  - channel-major q, k each ONE 4MB DMA (host-packed p-major).
  - load order per batch: qkt groups, then qc (out_b needs it first),
    then kc.
  - row/col-sum + reciprocal tiles consolidated to [128,4] (fewer
    semaphores -> shorter BSP epilogue; 2 reciprocals instead of 8).
  - the final store of the last batch is split in half so the end
    drain is ~1.5us instead of ~3us.
"""

import numpy as np

B, C, D = 16, 512, 4096
N_CORES = 8
B_PER_CORE = B // N_CORES  # 2
CC = C // 128  # 4 c-chunks
DC = D // 128  # 32 d-chunks
NG = D // 512  # 8 d-groups of 512 in the out phase
# d-major load group sizes (in 128-row d-chunks): a small first group so
# the first score matmuls start ~3us earlier during the DMA-bound fill.
GS = [2, 6, 8, 8, 8]
GCUM = [0, 2, 8, 16, 24, 32]
NGT = len(GS)
GQ = 8  # d-chunks in the final (cc-outer) group
ROWS = B_PER_CORE * C  # 1024

_COMPILED = {}


def _build():
    import concourse.mybir as mybir
    from concourse import bacc, tile

    f32 = mybir.dt.float32
    bf16 = mybir.dt.bfloat16
    AF = mybir.ActivationFunctionType

    nc = bacc.Bacc(None, target_bir_lowering=False)
    # qkt: d-major q and k interleaved per group g:
    #   row b*128+p, col g*(2*GW) + half*GW + j*C + c  (half 0 = q, 1 = k)
    #   holds element (d = 128*(g*GQ+j)+p, c) of q/k for batch b.
    # qc/kc: channel-major, p-major packed:
    #   row b*128+p, col cc*D + d  ->  q[cc*128+p, d] of batch b.
    qkt = nc.declare_dram_parameter(
        "qkt", [B_PER_CORE * 128, 2 * DC * C], bf16, isOutput=False
    )
    qc = nc.declare_dram_parameter("qc", [B_PER_CORE * 128, CC * D], bf16, isOutput=False)
    kc = nc.declare_dram_parameter("kc", [B_PER_CORE * 128, CC * D], bf16, isOutput=False)
    ident = nc.declare_dram_parameter("ident", [128, 128], bf16, isOutput=False)
    oa = nc.declare_dram_parameter("oa", [ROWS, D], bf16, isOutput=True)
    ob = nc.declare_dram_parameter("ob", [ROWS, D], bf16, isOutput=True)

    with tile.TileContext(nc) as tc:
        with (
            tc.tile_pool(name="const", bufs=1) as constp,
            tc.tile_pool(name="qk", bufs=1) as qk,
            tc.tile_pool(name="pp", bufs=1) as pp,
            tc.tile_pool(name="rp", bufs=1) as rp,
            tc.tile_pool(name="osb", bufs=3) as osb,
            tc.tile_pool(name="sps", bufs=1, space="PSUM") as sps,
            tc.tile_pool(name="stgps", bufs=4, space="PSUM") as stgps,
        ):
            idt = constp.tile([128, 128], bf16)
            nc.sync.dma_start(idt[:], ident[:])

            def load_qkt(b):
                """Everything rides the single SP HWDGE ring, issued in the
                order it becomes loadable, so DMA-sem recycle waits land on
                the (idle) Sync queue and never block ACT/DVE compute."""
                p0 = b * 128
                qkt_t = []
                for g in range(NGT):
                    w = 2 * GS[g] * C
                    o = 2 * GCUM[g] * C
                    a = qk.tile([128, w], bf16, tag=f"qkt{g}", name=f"qkt{g}")
                    nc.sync.dma_start(a[:], qkt[p0 : p0 + 128, o : o + w])
                    qkt_t.append(a)
                return qkt_t

            def load_c(b, dram, pfx):
                p0 = b * 128
                out = []
                for cc in range(CC):
                    a = qk.tile([128, D], bf16, tag=f"{pfx}{cc}", name=f"{pfx}{cc}")
                    nc.sync.dma_start(a[:], dram[p0 : p0 + 128, cc * D : (cc + 1) * D])
                    out.append(a)
                return out

            # HAM warm-up: ~44 junk matmuls on the identity keep the PE
            # busy through the DMA-bound fill so the real score matmuls
            # start at 2.4 GHz instead of 1.2.
            junk = stgps.tile([128, 512], f32, tag="st", name="junk")
            for _ in range(44):
                nc.tensor.matmul(junk[:, 0:128], idt[:], idt[:], start=True, stop=True)

            tiles = (load_qkt(0), load_c(0, qc, "qc"), load_c(0, kc, "kc"))
            for b in range(B_PER_CORE):
                qkt_t, qc_t, kc_t = tiles
                r0 = b * C

                # ---- scores: S_ce[cc] accumulates over 32 d-chunks ----
                s_ps = [
                    sps.tile([128, C], f32, tag=f"s{cc}", name=f"s{cc}")
                    for cc in range(CC)
                ]
                # groups 0..NGT-2: d-outer (stream-friendly while loads
                # land); last group: cc-outer so the four S banks complete
                # STAGGERED and the serial exp+rowsum chain on ACT overlaps
                # the score-matmul tail instead of stalling out_b.
                def qkt_slices(dc):
                    g = next(i for i in range(NGT) if GCUM[i] <= dc < GCUM[i + 1])
                    j = dc - GCUM[g]
                    kofs = GS[g] * C
                    return g, j * C, kofs
                for dc in range(DC - GQ):
                    g, jo, kofs = qkt_slices(dc)
                    mv = qkt_t[g][:, kofs + jo : kofs + jo + C]
                    for cc in range(CC):
                        nc.tensor.matmul(
                            s_ps[cc][:],
                            qkt_t[g][:, jo + cc * 128 : jo + (cc + 1) * 128],
                            mv,
                            start=(dc == 0),
                            stop=False,
                        )
                for cc in range(CC):
                    for dc in range(DC - GQ, DC):
                        g, jo, kofs = qkt_slices(dc)
                        nc.tensor.matmul(
                            s_ps[cc][:],
                            qkt_t[g][:, jo + cc * 128 : jo + (cc + 1) * 128],
                            qkt_t[g][:, kofs + jo : kofs + jo + C],
                            start=False,
                            stop=(dc == DC - 1),
                        )

                # next batch's qkt: issued here (ring position after this
                # batch's loads) so its WAR deps release during our scores.
                if b + 1 < B_PER_CORE:
                    next_qkt = load_qkt(b + 1)

                # ---- exp + row sums (direction A) ----
                rs_a = rp.tile([128, CC], f32, tag="rsa", name="rsa")
                ri_a = rp.tile([128, CC], f32, tag="ria", name="ria")
                p_ce = []
                for cc in range(CC):
                    p = pp.tile([128, C], bf16, tag=f"pce{cc}", name=f"pce{cc}")
                    nc.scalar.activation(
                        p[:],
                        s_ps[cc][:],
                        AF.Exp,
                        scale=1.0 / 64.0,
                        accum_out=rs_a[:, cc : cc + 1],
                    )
                    p_ce.append(p)
                nc.vector.reciprocal(ri_a[:], rs_a[:])

                # ---- transpose P -> P_ec + column sums (direction B) ----
                # t_ps reuses the S banks (tags s0-3): free after the exps.
                rs_b = rp.tile([128, CC], f32, tag="rsb", name="rsb")
                ri_b = rp.tile([128, CC], f32, tag="rib", name="rib")
                p_ec = []
                for ec in range(CC):
                    esl = slice(ec * 128, (ec + 1) * 128)
                    t_ps = sps.tile([128, C], bf16, tag=f"s{ec}", name=f"pt{ec}")
                    for cc in range(CC):
                        nc.tensor.transpose(
                            t_ps[:, cc * 128 : (cc + 1) * 128], p_ce[cc][:, esl], idt[:]
                        )
                    p = pp.tile([128, C], bf16, tag=f"pec{ec}", name=f"pec{ec}")
                    nc.scalar.activation(
                        p[:], t_ps[:], AF.Copy, accum_out=rs_b[:, ec : ec + 1]
                    )
                    p_ec.append(p)
                nc.vector.reciprocal(ri_b[:], rs_b[:])

                # ---- out_b = (P_ce.T @ q) * rinv_b ----
                # (before out_a: only needs P_ce, so it hides the P_ec
                # transpose/copy tail.)
                for ec in range(CC):
                    esl = slice(ec * 128, (ec + 1) * 128)
                    o_sb = osb.tile([128, D], bf16, tag="osb", name=f"ob{ec}")
                    for g in range(NG):
                        gsl = slice(g * 512, (g + 1) * 512)
                        o_ps = stgps.tile([128, 512], f32, tag="st", name="ob_ps")
                        for cc in range(CC):
                            nc.tensor.matmul(
                                o_ps[:],
                                p_ce[cc][:, esl],
                                qc_t[cc][:, g * 512 : (g + 1) * 512],
                                start=(cc == 0),
                                stop=(cc == CC - 1),
                            )
                        if g % 2 == 0:
                            nc.vector.tensor_scalar_mul(
                                o_sb[:, gsl], o_ps[:], ri_b[:, ec : ec + 1]
                            )
                        else:
                            nc.scalar.activation(
                                o_sb[:, gsl],
                                o_ps[:],
                                AF.Copy,
                                scale=ri_b[:, ec : ec + 1],
                            )
                    nc.sync.dma_start(
                        ob[r0 + ec * 128 : r0 + (ec + 1) * 128, :], o_sb[:]
                    )

                if b + 1 < B_PER_CORE:
                    next_qc = load_c(b + 1, qc, "qc")

                # ---- out_a = (P_ec.T @ k) * rinv_a ----
                for cc in range(CC):
                    csl = slice(cc * 128, (cc + 1) * 128)
                    o_sb = osb.tile([128, D], bf16, tag="osb", name=f"oa{cc}")
                    for g in range(NG):
                        gsl = slice(g * 512, (g + 1) * 512)
                        o_ps = stgps.tile([128, 512], f32, tag="st", name="oa_ps")
                        for ec in range(CC):
                            nc.tensor.matmul(
                                o_ps[:],
                                p_ec[ec][:, csl],
                                kc_t[ec][:, g * 512 : (g + 1) * 512],
                                start=(ec == 0),
                                stop=(ec == CC - 1),
                            )
                        last_tile = b == B_PER_CORE - 1 and cc == CC - 1
                        if g % 2 == 0 or last_tile:
                            nc.vector.tensor_scalar_mul(
                                o_sb[:, gsl], o_ps[:], ri_a[:, cc : cc + 1]
                            )
                        else:
                            nc.scalar.activation(
                                o_sb[:, gsl],
                                o_ps[:],
                                AF.Copy,
                                scale=ri_a[:, cc : cc + 1],
                            )
                        if last_tile:
                            # stream the final tile out per-group: short drain
                            nc.sync.dma_start(
                                oa[r0 + cc * 128 : r0 + (cc + 1) * 128, gsl],
                                o_sb[:, gsl],
                            )
                    if not (b == B_PER_CORE - 1 and cc == CC - 1):
                        orows = slice(r0 + cc * 128, r0 + (cc + 1) * 128)
                        nc.sync.dma_start(oa[orows, :], o_sb[:])

                if b + 1 < B_PER_CORE:
                    next_kc = load_c(b + 1, kc, "kc")
                    tiles = (next_qkt, next_qc, next_kc)

    nc.finalize()
    return nc


def _get_nc():
    if "nc" not in _COMPILED:
        _COMPILED["nc"] = _build()
    return _COMPILED["nc"]


def _make_in_maps(x1, x2):
    import ml_dtypes

    bf16 = ml_dtypes.bfloat16
    x1 = np.ascontiguousarray(x1, dtype=np.float32).reshape(B, C, D)
    x2 = np.ascontiguousarray(x2, dtype=np.float32).reshape(B, C, D)
    ident = np.eye(128, dtype=bf16)

    in_maps = []
    for i in range(N_CORES):
        sl = slice(i * B_PER_CORE, (i + 1) * B_PER_CORE)
        c1 = x1[sl].astype(bf16)  # [2, C, D]
        c2 = x2[sl].astype(bf16)
        # d-major [b, p, j, c] with d = 128*j + p, then groups of GQ j's
        # with q/k halves interleaved per group.
        t1 = c1.transpose(0, 2, 1).reshape(B_PER_CORE, DC, 128, C)
        t2 = c2.transpose(0, 2, 1).reshape(B_PER_CORE, DC, 128, C)
        # per group g: q chunks [GCUM[g]:GCUM[g+1]] then k chunks, p-major
        parts = []
        for g in range(NGT):
            for t in (t1, t2):
                blk = t[:, GCUM[g] : GCUM[g + 1]]  # [b, GS[g], 128, C]
                parts.append(
                    blk.transpose(0, 2, 1, 3).reshape(B_PER_CORE, 128, GS[g] * C)
                )
        qkt_h = np.ascontiguousarray(np.concatenate(parts, axis=2)).reshape(
            B_PER_CORE * 128, 2 * DC * C
        )
        # channel-major p-major: [b, p, cc, d]
        cc1 = c1.reshape(B_PER_CORE, CC, 128, D).transpose(0, 2, 1, 3)
        cc2 = c2.reshape(B_PER_CORE, CC, 128, D).transpose(0, 2, 1, 3)
        in_maps.append(
            {
                "qkt": qkt_h,
                "qc": np.ascontiguousarray(cc1).reshape(B_PER_CORE * 128, CC * D),
                "kc": np.ascontiguousarray(cc2).reshape(B_PER_CORE * 128, CC * D),
                "ident": ident,
            }
        )
    return in_maps


def kernel(x1: np.ndarray, x2: np.ndarray):
    from concourse.bass_utils import run_bass_kernel_spmd

    nc = _get_nc()
    in_maps = _make_in_maps(x1, x2)

    res = None
    for attempt in range(3):
        try:
            res = run_bass_kernel_spmd(nc, in_maps, list(range(N_CORES))).results
            break
        except Exception:
            if attempt == 2:
                raise
    assert res is not None

    outA = np.empty((B, C, 64, 64), dtype=np.float32)
    outB = np.empty((B, C, 64, 64), dtype=np.float32)
    for i in range(N_CORES):
        sl = slice(i * B_PER_CORE, (i + 1) * B_PER_CORE)
        outA[sl] = res[i]["oa"].astype(np.float32).reshape(B_PER_CORE, C, 64, 64)
        outB[sl] = res[i]["ob"].astype(np.float32).reshape(B_PER_CORE, C, 64, 64)
    return outA, outB
